# revision 35
# baseline (speedup 1.0000x reference)
"""CapsuleLayer (dynamic routing) Trainium2 kernel, v3.

x[32,2048,16] f32, W[64,2048,32,16] f32 ->
  u_hat = einsum('jidk,bik->bjid'); 3 dynamic-routing iterations
  (softmax over num_capsule j, s = sum_i c*u_hat, v = squash(s),
   logits += v.u_hat); returns v [32,64,32] f32.

Sharding: in_caps axis i split across 8 cores (I_loc=256/core); W shard is
SBUF-resident f16; only s-partials [32, 2048] f32 cross cores (one
AllReduce per iteration).

One continuous software pipeline across all four (r, bg) blocks; per ic
three phases: A1 (u-matmul + ACT evac of PSUM), A2 (prod = u*v on DVE,
d-tree split DVE lvl1 / Pool lvl2-5), B split over two steps: B1 (exp
on ACT, recip on DVE, c16 = e*zi on ACT via activation(scale=zi)), B2
one step later (cu on DVE, s-matmul) so the softmax chain never gates
cu.  Emission order per step keeps each in-order engine's stream
dependency-clean (DVE leads with prod so it never waits on the exp
chain; ACT leads with exp so softmax never queues behind 2us of
evacuations; c16 sits between the two evac halves; PE runs u-mm before
s-mm).  Collective + squash + v-expand
run PER batch-group, so iteration r+1's A2/B streams have their v long
before the pipeline reaches them and the r-boundary costs nothing; the
u-matmul stream never pauses.  r0 computes s0 in one pass with a
combined b=32 lhsT; W is loaded in 8 chunks interleaved on 3 DMA
queues so the r0 chain starts ~7us in.  Logits, s-partials and the
AllReduce payload are f16 (error budget checked: rel fro err 6.3e-4 vs
the 2e-2 gate).

TimelineSim (cost model): 509498 ns/core (baseline 664431, -23.3%).
Engine busy: DVE ~77% (wall: prod+cu at 2x f16 + lvl1), ACT ~70%,
Pool ~64%, PE ~53%; remaining idle is the serial ~50us W DMA at
startup (the cost model serializes all DMA) and the final drain.
"""

import numpy as np

B, I_FULL, K = 32, 2048, 16
J, D = 64, 32
N_CORES = 8
I_LOC = I_FULL // N_CORES   # 256
IC = I_LOC // 8             # 32 i-chunks of 8 i's
BG = 2                      # batch groups of 16
DJ = D * J                  # 2048
ROUTINGS = 3
EPS = 1e-7

KNOBS = dict(
    evac_dve=(0, 5),    # (num, den): (ic*2+h) % den < num -> DVE else ACT
    lvl1_split=64,   # j < split on DVE, rest on Pool (64 = all DVE)
    lvl2_dve=(0, 6),
    lvl4_dve=(0, 6),
    lvl5_dve=(0, 6),
    c16_eng="act",
    recip_act=False,
    lag_a2=2,           # A2 trails A1 (in ic steps)
    lag_b=2,            # B trails A2
    u16_bufs=6,
    prod_bufs=2,
    cu_bufs=2,
    l_f16=True,
)

_cache = {}


def _build_program(n_cores=N_CORES, collective=True, knobs=None):
    import concourse.bacc as bacc
    import concourse.bass as bass
    import concourse.tile as tile
    from concourse import mybir

    kn = dict(KNOBS)
    if knobs:
        kn.update(knobs)

    f32 = mybir.dt.float32
    f16 = mybir.dt.float16
    fL = f16 if kn["l_f16"] else f32

    nc = bacc.Bacc("TRN2", target_bir_lowering=False, debug=False,
                   num_devices=n_cores)

    wp_d = nc.dram_tensor("wp", [128, IC, DJ], f16, kind="ExternalInput")
    xbd_d = nc.dram_tensor("xbd", [128, IC, BG, 128], f16,
                           kind="ExternalInput")
    ones_d = nc.dram_tensor("ones", [128, 16], f16, kind="ExternalInput")
    xsum_d = nc.dram_tensor("xsum", [128, IC, B], f16, kind="ExternalInput")
    v_out = nc.dram_tensor("v_out", [B, D, J], f32, kind="ExternalOutput")

    def bcast(ap, axis, n):
        new = list(ap.ap)
        new.insert(axis, [0, n])
        return bass.AP(tensor=ap.tensor, offset=ap.offset, ap=new)

    with tile.TileContext(nc) as tc:
        with (
            tc.tile_pool(name="res", bufs=1) as res,
            tc.tile_pool(name="work", bufs=2) as work,
            tc.tile_pool(name="small", bufs=4) as small,
            tc.tile_pool(name="sq", bufs=1) as sq,
            tc.tile_pool(name="pu", bufs=2, space="PSUM") as pu,
            tc.tile_pool(name="ps", bufs=1, space="PSUM") as ps,
            tc.tile_pool(name="dram", bufs=1, space="DRAM") as dram,
        ):
            # ---------- DRAM scratch ----------
            s_part = dram.tile([B, DJ], f16, name="s_part")
            s_fulls = {(r, bg): dram.tile([16, D, J], f16,
                                          name=f"s_full{r}_{bg}",
                                          addr_space="Shared")
                       for r in range(ROUTINGS) for bg in range(BG)}
            v_dram = dram.tile([B, D, J], f16, name="v_dram")

            # ---------- resident SBUF ----------
            wp = res.tile([128, IC, DJ], f16, name="wp_sb")        # 128KB/p
            ones = res.tile([128, 16], f16, name="ones_sb")
            xsum = res.tile([128, IC, B], f16, name="xsum_sb")     # 2KB/p
            nc.sync.dma_start(out=xsum, in_=xsum_d.ap())
            nc.sync.dma_start(out=ones, in_=ones_d.ap())
            # wp split into 8 chunks round-robined over 3 DMA queues so the
            # r0 matmul chain starts early and consumes chunks in order.
            qs = [nc.scalar, nc.gpsimd, nc.sync]
            icq = IC // 8
            for chunk in range(8):
                qs[chunk % 3].dma_start(
                    out=wp[:, chunk * icq:(chunk + 1) * icq, :],
                    in_=wp_d.ap()[:, chunk * icq:(chunk + 1) * icq, :])

            L = res.tile([128, BG, IC, J], fL, name="L_sb")

            vexp_t = {}

            def load_vexp(r, bg):
                vx = work.tile([128, D, J], f16, name="vexp", bufs=2)
                for rep in range(8):
                    nc.sync.dma_start(
                        out=vx[rep * 16:(rep + 1) * 16, :, :],
                        in_=v_dram[bg * 16:(bg + 1) * 16, :, :])
                vexp_t[(r, bg)] = vx

            xbd_t = {}

            def load_xbd(r, bg):
                for q in range(4):
                    xq = work.tile([128, IC // 4, 128], f16, name="xbd",
                                   bufs=2)
                    nc.sync.dma_start(
                        out=xq,
                        in_=xbd_d.ap()[:, q * (IC // 4):(q + 1) * (IC // 4),
                                       bg, :])
                    xbd_t[(r, bg, q)] = xq

            def pick(knob, idx):
                num, den = kn[knob]
                return (idx % den) < num

            # ================= r0: s0 via collapsed lhsT =================
            s_ps32 = ps.tile([B, DJ], f32, name="s_ps32", tag="sps")
            for ic in range(IC):
                for q in range(4):
                    nc.tensor.matmul(
                        out=s_ps32[:, q * 512:(q + 1) * 512],
                        lhsT=xsum[:, ic, :],
                        rhs=wp[:, ic, q * 512:(q + 1) * 512],
                        start=(ic == 0), stop=(ic == IC - 1),
                        skip_group_check=True)

            # ============ pipeline phases ============
            stash_u = {}
            stash_e = {}
            stash_t = {}
            s_ps_t = {}

            def phase_a1_h(r, bg, ic, h):
                if h == 0:
                    u16 = work.tile([128, D, J], f16, name="u16",
                                    bufs=kn["u16_bufs"])
                    stash_u[(r, bg, ic)] = u16
                else:
                    u16 = stash_u[(r, bg, ic)]
                u_ps = pu.tile([128, DJ // 2], f32, name="u_ps")
                for q in range(2):
                    nc.tensor.matmul(
                        out=u_ps[:, q * 512:(q + 1) * 512],
                        lhsT=xbd_t[(r, bg, ic // (IC // 4))][
                            :, ic % (IC // 4), :],
                        rhs=wp[:, ic, h * (DJ // 2) + q * 512:
                               h * (DJ // 2) + (q + 1) * 512],
                        start=True, stop=True)
                dst = u16[:, h * 16:(h + 1) * 16, :].rearrange(
                    "p a b -> p (a b)")
                if pick("evac_dve", ic * 2 + h):
                    nc.vector.tensor_copy(out=dst, in_=u_ps)
                else:
                    nc.scalar.copy(out=dst, in_=u_ps)

            def phase_a2_prod(r, bg, ic):
                u16 = stash_u[(r, bg, ic)]
                prod = work.tile([128, D, J], f16, name="prod",
                                 bufs=kn["prod_bufs"])
                nc.vector.tensor_mul(out=prod, in0=u16, in1=vexp_t[(r, bg)])
                stash_t[(r, bg, ic)] = prod

            def phase_a2_tree1(r, bg, ic):
                prod = stash_t[(r, bg, ic)]
                js = kn["lvl1_split"]   # j-split: DVE [0:js), Pool [js:J)
                if js >= J:
                    nc.vector.tensor_add(out=prod[:, 0:16, :],
                                         in0=prod[:, 0:16, :],
                                         in1=prod[:, 16:32, :])
                else:
                    if js > 0:
                        nc.vector.tensor_add(out=prod[:, 0:16, 0:js],
                                             in0=prod[:, 0:16, 0:js],
                                             in1=prod[:, 16:32, 0:js])
                    nc.gpsimd.tensor_add(out=prod[:, 0:16, js:],
                                         in0=prod[:, 0:16, js:],
                                         in1=prod[:, 16:32, js:])
                eng2 = nc.vector if pick("lvl2_dve", ic) else nc.gpsimd
                eng2.tensor_add(out=prod[:, 0:8, :], in0=prod[:, 0:8, :],
                                in1=prod[:, 8:16, :])
                nc.gpsimd.tensor_add(out=prod[:, 0:4, :], in0=prod[:, 0:4, :],
                                     in1=prod[:, 4:8, :])

            def phase_a2_tree2(r, bg, ic):
                prod = stash_t.pop((r, bg, ic))
                eng4 = nc.vector if pick("lvl4_dve", ic) else nc.gpsimd
                eng4.tensor_add(out=prod[:, 0:2, :], in0=prod[:, 0:2, :],
                                in1=prod[:, 2:4, :])
                eng5 = nc.vector if pick("lvl5_dve", ic) else nc.gpsimd
                if r == 1:
                    eng5.tensor_add(out=L[:, bg, ic, :],
                                    in0=prod[:, 0, :], in1=prod[:, 1, :])
                else:
                    ltmp = small.tile([128, J], fL, name="ltmp")
                    eng5.tensor_add(out=ltmp, in0=prod[:, 0, :],
                                    in1=prod[:, 1, :])
                    eng5.tensor_add(out=L[:, bg, ic, :],
                                    in0=L[:, bg, ic, :], in1=ltmp)

            def phase_b_exp(r, bg, ic):
                e_t = small.tile([128, J], f32, name="e_t")
                z_t = small.tile([128, 1], f32, name="z_t")
                nc.scalar.activation(
                    out=e_t, in_=L[:, bg, ic, :],
                    func=mybir.ActivationFunctionType.Exp, accum_out=z_t)
                stash_e[(r, bg, ic)] = (e_t, z_t)

            def phase_b_recip(r, bg, ic):
                e_t, z_t = stash_e.pop((r, bg, ic))
                zi = small.tile([128, 1], f32, name="zi")
                if kn["recip_act"]:
                    nc.scalar.activation(
                        out=zi, in_=z_t,
                        func=mybir.ActivationFunctionType.Reciprocal)
                else:
                    nc.vector.reciprocal(out=zi, in_=z_t)
                stash_e[(r, bg, ic, "z")] = (e_t, zi)

            def phase_b_c16(r, bg, ic):
                e_t, zi = stash_e.pop((r, bg, ic, "z"))
                c16 = small.tile([128, J], f16, name="c16", bufs=3)
                eng = kn["c16_eng"]
                if eng == "act":
                    nc.scalar.activation(
                        out=c16, in_=e_t,
                        func=mybir.ActivationFunctionType.Copy,
                        scale=zi[:])
                elif eng == "pool":
                    nc.gpsimd.tensor_scalar_mul(out=c16, in0=e_t, scalar1=zi)
                else:
                    nc.vector.tensor_scalar_mul(out=c16, in0=e_t, scalar1=zi)
                stash_e[(r, bg, ic, "c")] = c16

            def phase_b_tail(r, bg, ic):
                c16 = stash_e.pop((r, bg, ic, "c"))
                u16 = stash_u.pop((r, bg, ic))
                cu = work.tile([128, D, J], f16, name="cu",
                               bufs=kn["cu_bufs"])
                c_b = bcast(c16[:], 1, D)
                nc.vector.tensor_mul(out=cu, in0=u16, in1=c_b)
                rflat = cu[:].rearrange("p a b -> p (a b)")
                s_ps = s_ps_t[(r, bg)]
                for q in range(4):
                    nc.tensor.matmul(
                        out=s_ps[:, q * 512:(q + 1) * 512],
                        lhsT=ones,
                        rhs=rflat[:, q * 512:(q + 1) * 512],
                        start=(ic == 0), stop=(ic == IC - 1),
                        skip_group_check=True)

            # ---------- per-bg collective + squash + vexp reload ----------
            def squash_and_out(r, bg, s_src=None):
                """Squash rows [bg*16, bg*16+16).  s_src: PSUM tile to evac
                (None for r0, which uses the pre-evacuated s_part)."""
                sl = slice(bg * 16, (bg + 1) * 16)
                if s_src is not None:
                    s_ev = sq.tile([16, DJ], f16, name="s_ev", tag="sev")
                    nc.scalar.copy(out=s_ev, in_=s_src)
                    nc.sync.dma_start(out=s_part[sl, :], in_=s_ev)
                if collective:
                    nc.gpsimd.collective_compute(
                        "AllReduce", mybir.AluOpType.add,
                        replica_groups=[list(range(n_cores))],
                        ins=[s_part[sl, :].opt()],
                        outs=[s_fulls[(r, bg)][:].rearrange(
                            "b d j -> b (d j)").opt()])
                else:
                    nc.sync.dma_start(
                        out=s_fulls[(r, bg)][:],
                        in_=s_part[sl, :].rearrange("b (d j) -> b d j", d=D))
                s_sb = sq.tile([16, D, J], f16, name="s_sb", tag="sev2")
                nc.sync.dma_start(out=s_sb, in_=s_fulls[(r, bg)][:])
                s2 = sq.tile([16, D, J], f16, name="s2", tag="sev")
                JP = 16   # j-range handled by Pool
                nc.vector.tensor_mul(out=s2[:, :, JP:], in0=s_sb[:, :, JP:],
                                     in1=s_sb[:, :, JP:])
                nc.gpsimd.tensor_mul(out=s2[:, :, 0:JP], in0=s_sb[:, :, 0:JP],
                                     in1=s_sb[:, :, 0:JP])
                w = D
                while w > 1:
                    hw = w // 2
                    nc.vector.tensor_add(out=s2[:, 0:hw, JP:],
                                         in0=s2[:, 0:hw, JP:],
                                         in1=s2[:, hw:w, JP:])
                    nc.gpsimd.tensor_add(out=s2[:, 0:hw, 0:JP],
                                         in0=s2[:, 0:hw, 0:JP],
                                         in1=s2[:, hw:w, 0:JP])
                    w = hw
                n_t = s2[:, 0:1, :]
                eps_t = sq.tile([16, 1], f32, name="eps_t")
                nc.vector.memset(eps_t, EPS)
                sqr = sq.tile([16, 1, J], f32, name="sqr")
                nc.scalar.activation(out=sqr, in_=n_t[:],
                                     func=mybir.ActivationFunctionType.Sqrt,
                                     bias=eps_t[:], scale=1.0)
                onep = sq.tile([16, 1, J], f32, name="onep")
                nc.scalar.add(out=onep, in_=n_t[:], add=1.0)
                nc.vector.tensor_mul(out=onep, in0=onep, in1=sqr)
                rec = sq.tile([16, 1, J], f32, name="rec")
                nc.vector.reciprocal(out=rec, in_=onep)
                scl = sq.tile([16, 1, J], f16, name="scl")
                nc.vector.tensor_mul(out=scl, in0=n_t[:], in1=rec)
                scl_b = bcast(scl[:, 0, :], 1, D)
                if r == ROUTINGS - 1:
                    scl_h = bcast(scl[:, 0, :], 1, D // 2)
                    for dh in range(2):
                        v_sb = sq.tile([16, D // 2, J], f32, name="v_sb",
                                       tag="s2b")
                        nc.vector.tensor_mul(
                            out=v_sb, in0=s_sb[:, dh * 16:(dh + 1) * 16, :],
                            in1=scl_h)
                        nc.sync.dma_start(
                            out=v_out.ap()[sl, dh * 16:(dh + 1) * 16, :],
                            in_=v_sb)
                else:
                    v16 = sq.tile([16, D, J], f16, name="v16", tag="s2b")
                    nc.vector.tensor_mul(out=v16, in0=s_sb, in1=scl_b)
                    nc.sync.dma_start(out=v_dram[sl, :, :], in_=v16)
                    load_vexp(r + 1, bg)

            # ================= merged global pipeline =================
            la, lb = kn["lag_a2"], kn["lag_b"]
            gap = kn.get("rgap", 0)     # extra A2/B slack at the r boundary
            blocks = [(1, 0), (1, 1), (2, 0), (2, 1)]
            seq = [(r, bg, ic) for (r, bg) in blocks for ic in range(IC)]
            A1_at, A2_at, B1_at, B2_at, T2_at = {}, {}, {}, {}, {}
            for idx, (r, bg, ic) in enumerate(seq):
                sh = gap if r == 2 else 0
                A1_at[idx] = (r, bg, ic)
                A2_at[idx + la + sh] = (r, bg, ic)
                T2_at[idx + la + sh + 1] = (r, bg, ic)
                B1_at[idx + la + lb + sh] = (r, bg, ic)
                B2_at[idx + la + lb + sh + 1] = (r, bg, ic)
            nsteps = len(seq) + la + lb + gap + 2

            load_xbd(1, 0)
            s_ps_t[(1, 0)] = ps.tile([16, DJ], f32, name="s_ps", tag="sps")

            for gt in range(nsteps):
                sb1 = B1_at.get(gt)
                sb2 = B2_at.get(gt)
                sa2 = A2_at.get(gt)
                st2 = T2_at.get(gt)
                sa1 = A1_at.get(gt)
                if sb1 is not None:
                    phase_b_exp(*sb1)
                if sa2 is not None:
                    phase_a2_prod(*sa2)
                if sa1 is not None:
                    phase_a1_h(*sa1, 0)
                if sb1 is not None:
                    phase_b_recip(*sb1)
                    phase_b_c16(*sb1)
                if sa1 is not None:
                    phase_a1_h(*sa1, 1)
                if sb2 is not None:
                    phase_b_tail(*sb2)
                if sa2 is not None:
                    phase_a2_tree1(*sa2)
                if st2 is not None:
                    phase_a2_tree2(*st2)
                if sb2 is not None:
                    r, bg, ic = sb2
                    if ic == IC - 1:
                        squash_and_out(r, bg, s_src=s_ps_t.pop((r, bg)))
                # prepare next block's inputs well before they're needed
                if sa1 is not None:
                    r, bg, ic = sa1
                    if ic == IC - 8:
                        bi = gt // IC
                        if bi + 1 < 4:
                            nr, nbg = blocks[bi + 1]
                            load_xbd(nr, nbg)
                            s_ps_t[(nr, nbg)] = ps.tile(
                                [16, DJ], f32, name="s_ps", tag="sps")
                if gt == 1:
                    # r0 epilogue: runs while (r1,bg0) u-matmuls fill the pipe
                    s_ev32 = sq.tile([B, DJ], f16, name="s_ev32", tag="sev")
                    nc.scalar.copy(out=s_ev32, in_=s_ps32)
                    nc.sync.dma_start(out=s_part[:], in_=s_ev32)
                    squash_and_out(0, 0)
                    squash_and_out(0, 1)

    nc.compile()
    return nc


def _pack_inputs(x, W):
    """Host-side packing of per-core kernel inputs."""
    in_maps = []
    base = np.zeros((128, 16), np.float32)
    for i in range(8):
        base[i * 16:(i + 1) * 16] = np.eye(16)
    ones = base.astype(np.float16)                      # [128, 16]
    for c in range(N_CORES):
        sl = slice(c * I_LOC, (c + 1) * I_LOC)
        Wc = W[:, sl]                                   # [J, 256, D, K]
        wp = Wc.reshape(J, IC, 8, D, K).transpose(2, 4, 1, 3, 0)
        # wp: [i, k, ic, d, j] -> [(i k)=128, IC, DJ]
        wp = np.ascontiguousarray(wp).reshape(128, IC, DJ).astype(np.float16)
        xc = x[:, sl]                                   # [B, 256, K]
        xcol = xc.reshape(BG, 16, IC, 8, K).transpose(3, 4, 2, 0, 1)
        # xcol: [i, k, ic, bg, b]
        xbd = np.zeros((8, K, IC, BG, 8, 16), np.float32)
        for i in range(8):
            xbd[i, :, :, :, i, :] = xcol[i]
        xbd = xbd.reshape(128, IC, BG, 128)
        xsum = (xcol / 64.0).reshape(128, IC, B)        # b = (bg, b16)
        in_maps.append({"wp": wp, "xbd": xbd.astype(np.float16),
                        "xsum": xsum.astype(np.float16),
                        "ones": ones})
    return in_maps


def kernel(x, W, trace=False, trace_kwargs=None):
    from concourse import bass_utils
    if "nc" not in _cache:
        _cache["nc"] = _build_program()
    nc = _cache["nc"]
    in_maps = _pack_inputs(np.asarray(x, np.float32),
                           np.asarray(W, np.float32))
    res = bass_utils.run_bass_kernel_spmd(
        nc, in_maps, core_ids=list(range(N_CORES)), trace=trace,
        **(trace_kwargs or {}))
    if trace:
        _cache["last_results"] = res
    v = res.results[0]["v_out"]          # [B, D, J]
    return np.ascontiguousarray(v.transpose(0, 2, 1))  # [B, J, D]
